# revision 1
# baseline (speedup 1.0000x reference)
"""Bidirectional LSTM kernel for 8 trn2 NeuronCores.

Problem: T=512, B=64, I=512, H=512, fp32 in/out.
  out[t] = concat(h_f[t], h_b[t]), 2 independent LSTM scans (fwd / bwd).

Sharding: core c in 0..7 -> direction d = c//4 (0=fwd, 1=bwd), batch shard
  bs = 16*(c%4) .. +16.  Backward direction handled purely by host-side data
  prep (time-reversed input; output un-reversed on host).

Per-core plan (SPMD; all cores run the identical program on different data):
  Phase 1: G[t,b,:] = x[t,b,:] @ W_ihT_r + bias   (gate-permuted, bf16 in DRAM)
     stationary = x chunk (128 (t,b) rows), moving = W_ihT (bf16). 64 chunks.
  Phase 2: 512 sequential steps. gates = hT.T @ W_hhT_r (+G via DVE add).
     stationary = hT [128,16] bf16, moving = W_hhT fp32r ([128,512] slices).
     Gate columns permuted so chunk n (512 cols) = [i|f|o|g] for hidden slice
     [128n,128n+128) -> each chunk's elementwise is self-contained.
     h chunk [16,128] fp32 -> PE transpose -> psum -> copy to hT bf16.
"""

import numpy as np
import ml_dtypes

T, B, I, H = 512, 64, 512, 512
BL = 16           # batch per core
NC = 8            # cores
GB = 4            # steps per G-load / out-store batch
G4 = 4 * H        # 2048

LAST_RUN_INFO = {}


def _gate_perm():
    # reference gate order after split: i, f, g, o (rows 0:512, 512:1024, 1024:1536, 1536:2048)
    # chunk n layout: [i_n | f_n | o_n | g_n], each 128 wide, hidden slice 128n..128n+128
    idx = []
    for n in range(4):
        s = 128 * n
        idx.extend(range(0 + s, 0 + s + 128))        # i
        idx.extend(range(512 + s, 512 + s + 128))    # f
        idx.extend(range(1536 + s, 1536 + s + 128))  # o
        idx.extend(range(1024 + s, 1024 + s + 128))  # g
    return np.array(idx)


def _build_program():
    import concourse.bass as bass
    import concourse.tile as tile
    from concourse import bacc, mybir

    nc = bacc.Bacc()
    f32, f32r, bf16 = mybir.dt.float32, mybir.dt.float32r, mybir.dt.bfloat16

    xT = nc.declare_dram_parameter("xT", [I, T * BL], bf16, isOutput=False)
    wih = nc.declare_dram_parameter("wih", [I, G4], bf16, isOutput=False)
    whh = nc.declare_dram_parameter("whh", [H, G4], f32, isOutput=False)
    bias128 = nc.declare_dram_parameter("bias128", [128, G4], f32, isOutput=False)
    eye16 = nc.declare_dram_parameter("eye16", [16, 16], f32, isOutput=False)
    out_loc = nc.declare_dram_parameter("out_loc", [T, BL, H], f32, isOutput=True)

    G = nc.dram_tensor("G_scratch", [T, BL, G4], bf16)

    SIG = mybir.ActivationFunctionType.Sigmoid
    TANH = mybir.ActivationFunctionType.Tanh

    with tile.TileContext(nc) as tc:
        from contextlib import ExitStack
        with ExitStack() as ctx:
            singles = ctx.enter_context(tc.tile_pool(name="singles", bufs=1))

            wih_sb = singles.tile([128, 4, G4], bf16)
            nc.sync.dma_start(out=wih_sb, in_=wih.rearrange("(k i) c -> i k c", i=128))
            whh_sb = singles.tile([128, 4, G4], f32r)
            nc.gpsimd.dma_start(out=whh_sb, in_=whh.rearrange("(k i) c -> i k c", i=128))
            bias_sb = singles.tile([128, G4], f32)
            nc.sync.dma_start(out=bias_sb, in_=bias128[:, :])
            eye_sb = singles.tile([16, 16], f32)
            nc.sync.dma_start(out=eye_sb, in_=eye16[:, :])
            eye_r = singles.tile([16, 16], f32r)
            nc.gpsimd.dma_start(out=eye_r, in_=eye16[:, :])

            c_st = singles.tile([BL, H], f32)
            nc.vector.memset(c_st, 0.0)

            # ---------------- Phase 1: G = x @ W_ihT + bias ----------------
            xTr = xT.rearrange("(k i) c -> i k c", i=128)
            with (
                tc.tile_pool(name="p1psum", bufs=8, space="PSUM") as pp1,
                tc.tile_pool(name="p1x", bufs=3) as xq,
                tc.tile_pool(name="p1g", bufs=6) as gq,
            ):
                for rc in range(T * BL // 128):
                    x_sb = xq.tile([128, 4, 128], bf16, tag="x")
                    nc.sync.dma_start(out=x_sb, in_=xTr[:, :, 128 * rc:128 * (rc + 1)])
                    psums = []
                    for n in range(4):
                        psums.append(pp1.tile([128, 512], f32, tag="pg", name=f"pg_{rc}_{n}"))
                    for k in range(4):
                        for n in range(4):
                            nc.tensor.matmul(
                                psums[n], x_sb[:, k, :],
                                wih_sb[:, k, 512 * n:512 * (n + 1)],
                                start=(k == 0), stop=(k == 3),
                            )
                    for n in range(4):
                        gout = gq.tile([128, 512], bf16, tag="gout")
                        nc.vector.tensor_add(gout, psums[n], bias_sb[:, 512 * n:512 * (n + 1)])
                        nc.sync.dma_start(
                            out=G[8 * rc:8 * (rc + 1), :, 512 * n:512 * (n + 1)],
                            in_=gout,
                        )

            # ---------------- Phase 2: recurrence ----------------
            with (
                tc.tile_pool(name="pgate", bufs=4, space="PSUM") as pg_pool,
                tc.tile_pool(name="ptr", bufs=4, space="PSUM") as pt_pool,
                tc.tile_pool(name="gload", bufs=2) as gl_pool,
                tc.tile_pool(name="obuf", bufs=2) as ob_pool,
                tc.tile_pool(name="work", bufs=3) as wk,
                tc.tile_pool(name="hTp", bufs=2) as hp,
            ):
                hT_prev = None

                gbuf = None
                obuf = None
                for s in range(T):
                    tl = s % GB
                    if tl == 0:
                        gbuf = gl_pool.tile([BL, GB * G4], f32r, tag="gb")
                        nc.gpsimd.dma_start(
                            out=gbuf,
                            in_=G[s:s + GB].rearrange("t b c -> b t c"),
                        )
                        obuf = ob_pool.tile([BL, GB * H], f32, tag="ob")
                    hT_cur = hp.tile([128, 64], f32r, tag="hT")
                    for n in range(4):
                        gsl = gbuf[:, tl * G4 + 512 * n: tl * G4 + 512 * (n + 1)]
                        pg = pg_pool.tile([BL, 512], f32, tag="pg2", name=f"pg2_{s}_{n}")
                        if s == 0:
                            nc.tensor.matmul(pg, eye_r, gsl, start=True, stop=True)
                            src_ap = pg
                        elif n < 2:
                            nc.tensor.matmul(pg, eye_r, gsl, start=True, stop=False)
                            for k in range(4):
                                nc.tensor.matmul(
                                    pg, hT_prev[:, 16 * k:16 * (k + 1)],
                                    whh_sb[:, k, 512 * n:512 * (n + 1)],
                                    start=False, stop=(k == 3),
                                )
                            src_ap = pg
                        else:
                            for k in range(4):
                                nc.tensor.matmul(
                                    pg, hT_prev[:, 16 * k:16 * (k + 1)],
                                    whh_sb[:, k, 512 * n:512 * (n + 1)],
                                    start=(k == 0), stop=(k == 3),
                                )
                            pre = wk.tile([BL, 512], f32, tag="pre")
                            nc.vector.tensor_add(pre, pg, gsl.bitcast(f32))
                            src_ap = pre
                        sig = wk.tile([BL, 384], f32, tag="sig")
                        nc.scalar.activation(out=sig, in_=src_ap[:, 0:384], func=SIG)
                        gt = wk.tile([BL, 128], f32, tag="gt")
                        nc.scalar.activation(out=gt, in_=src_ap[:, 384:512], func=TANH)
                        cs = c_st[:, 128 * n:128 * (n + 1)]
                        t1 = wk.tile([BL, 128], f32, tag="t1")
                        nc.gpsimd.tensor_mul(t1, sig[:, 128:256], cs)
                        t2 = wk.tile([BL, 128], f32, tag="t2")
                        nc.gpsimd.tensor_mul(t2, sig[:, 0:128], gt)
                        nc.vector.tensor_add(cs, t1, t2)
                        thc = wk.tile([BL, 128], f32, tag="thc")
                        nc.scalar.activation(out=thc, in_=cs, func=TANH)
                        hslot = obuf[:, tl * H + 128 * n: tl * H + 128 * (n + 1)]
                        nc.vector.tensor_mul(hslot, sig[:, 256:384], thc)
                        ptr = pt_pool.tile([128, 16], f32, tag="pt")
                        nc.tensor.transpose(ptr, hslot, eye_sb)
                        nc.vector.tensor_copy(out=hT_cur[:, 16 * n:16 * (n + 1)], in_=ptr)
                    hT_prev = hT_cur
                    if tl == GB - 1:
                        nc.sync.dma_start(
                            out=out_loc[s - GB + 1:s + 1].rearrange("t b h -> b t h"),
                            in_=obuf,
                        )

    nc.compile()
    return nc


_NC_CACHE = []


def kernel(**inputs):
    from concourse.bass_utils import run_bass_kernel_spmd
    import os

    X = np.asarray(inputs["input"], np.float32)
    perm = _gate_perm()
    eye = np.eye(16, dtype=np.float32)

    in_maps = []
    for c in range(NC):
        d = c // 4
        bs = BL * (c % 4)
        sfx = "f" if d == 0 else "b"
        Wih = np.asarray(inputs[f"W_ih_{sfx}"], np.float32)[perm]
        Whh = np.asarray(inputs[f"W_hh_{sfx}"], np.float32)[perm]
        bias = (np.asarray(inputs[f"b_ih_{sfx}"], np.float32)
                + np.asarray(inputs[f"b_hh_{sfx}"], np.float32))[perm]
        Xc = X[:, bs:bs + BL, :] if d == 0 else X[::-1, bs:bs + BL, :]
        in_maps.append({
            "xT": np.ascontiguousarray(Xc.transpose(2, 0, 1).reshape(I, T * BL)).astype(ml_dtypes.bfloat16),
            "wih": np.ascontiguousarray(Wih.T).astype(ml_dtypes.bfloat16),
            "whh": np.ascontiguousarray(Whh.T).astype(np.float32),
            "bias128": np.ascontiguousarray(np.broadcast_to(bias, (128, G4))).astype(np.float32),
            "eye16": eye,
            "out_loc": np.zeros((T, BL, H), np.float32),
        })
    in_maps = [{k: v for k, v in m.items() if k != "out_loc"} for m in in_maps]

    if not _NC_CACHE:
        _NC_CACHE.append(_build_program())
    nc = _NC_CACHE[0]

    trace = bool(int(os.environ.get("LSTM_TRACE", "0")))
    res = run_bass_kernel_spmd(nc, in_maps, list(range(NC)), trace=trace)
    LAST_RUN_INFO.clear()
    LAST_RUN_INFO["exec_time_ns"] = res.exec_time_ns
    LAST_RUN_INFO["profile_json"] = getattr(res, "profile_json", None)

    out = np.empty((T, B, 2 * H), np.float32)
    for c in range(NC):
        d = c // 4
        bs = BL * (c % 4)
        r = res.results[c]["out_loc"]
        if d == 0:
            out[:, bs:bs + BL, 0:H] = r
        else:
            out[:, bs:bs + BL, H:2 * H] = r[::-1]
    return out



# revision 4
# speedup vs baseline: 1.1618x; 1.1618x over previous
"""Bidirectional LSTM kernel for 8 trn2 NeuronCores.

Problem: T=512, B=64, I=512, H=512, fp32 in/out.
  out[t] = concat(h_f[t], h_b[t]), 2 independent LSTM scans (fwd / bwd).

Sharding: core c in 0..7 -> direction d = c//4 (0=fwd, 1=bwd), batch shard
  bs = 16*(c%4) .. +16.  Backward direction handled purely by host-side data
  prep (time-reversed input; output un-reversed on host).

Per-core plan (SPMD; all cores run the identical program on different data):
  Phase 1: G[t,b,:] = x[t,b,:] @ W_ihT_r + bias   (gate-permuted, bf16 in DRAM)
     stationary = x chunk (128 (t,b) rows), moving = W_ihT (bf16). 64 chunks.
  Phase 2: 512 sequential steps, per step:
     - 4 PSUM banks (chunk-major, layout [i|f|o|g] per 128-hidden chunk),
       ping-pong allocated so next step's matmuls prefetch under this step's
       elementwise.
     - G injected into PSUM via eye-matmul (bf16 moving).
     - recurrent matmuls in fp8e4m3 DoubleRow (2 k-tiles per pass): 8 matmuls
       of [128,2,16] x [128,2,512] per step.
     - g-gate columns pre-scaled x2 on host so tanh(g) = 2*sigmoid(g*2)-1,
       letting one sigmoid cover all 4 gates of a pair.
     - elementwise in 2 chunk-pairs: sigmoid (ACT) -> v=(sg-0.5)*si (DVE STT)
       / t1=sf*c (POOL) -> c=(2v+t1) (DVE STT) -> tanh_c (ACT) -> h=so*th
       (POOL, bf16 out) -> 2 PE transposes -> fp8 cast (DVE) into hT pair.
"""

import numpy as np
import ml_dtypes

T, B, I, H = 512, 64, 512, 512
BL = 16           # batch per core
NC = 8            # cores
GB = 4            # steps per G-load / out-store batch
G4 = 4 * H        # 2048

LAST_RUN_INFO = {}


def _gate_perm():
    # reference gate order after split: i, f, g, o (rows 0:512, 512:1024, 1024:1536, 1536:2048)
    # chunk n layout: [i_n | f_n | o_n | g_n], each 128 wide, hidden slice 128n..128n+128
    idx = []
    for n in range(4):
        s = 128 * n
        idx.extend(range(0 + s, 0 + s + 128))        # i
        idx.extend(range(512 + s, 512 + s + 128))    # f
        idx.extend(range(1536 + s, 1536 + s + 128))  # o
        idx.extend(range(1024 + s, 1024 + s + 128))  # g
    return np.array(idx)


def _g_scale():
    # x2 on the g-gate columns (384:512 of each 512 chunk) for the
    # tanh(g) = 2*sigmoid(2g)-1 fold
    s = np.ones(G4, np.float32)
    for n in range(4):
        s[512 * n + 384:512 * (n + 1)] = 2.0
    return s


def _build_program():
    import concourse.bass as bass
    import concourse.tile as tile
    from concourse import bacc, mybir

    nc = bacc.Bacc()
    f32, bf16 = mybir.dt.float32, mybir.dt.bfloat16
    fp8 = mybir.dt.float8e4
    DR = mybir.MatmulPerfMode.DoubleRow

    xT = nc.declare_dram_parameter("xT", [I, T * BL], bf16, isOutput=False)
    wih = nc.declare_dram_parameter("wih", [I, G4], bf16, isOutput=False)
    whh8 = nc.declare_dram_parameter("whh8", [H, G4], fp8, isOutput=False)
    bias128 = nc.declare_dram_parameter("bias128", [128, G4], f32, isOutput=False)
    eye16 = nc.declare_dram_parameter("eye16", [16, 16], bf16, isOutput=False)
    out_loc = nc.declare_dram_parameter("out_loc", [T, BL, H], bf16, isOutput=True)

    G = nc.dram_tensor("G_scratch", [T, BL, G4], bf16)

    SIG = mybir.ActivationFunctionType.Sigmoid
    TANH = mybir.ActivationFunctionType.Tanh
    MUL = mybir.AluOpType.mult
    ADD = mybir.AluOpType.add
    SUB = mybir.AluOpType.subtract

    with tile.TileContext(nc) as tc:
        from contextlib import ExitStack
        with ExitStack() as ctx:
            singles = ctx.enter_context(tc.tile_pool(name="singles", bufs=1))

            wih_sb = singles.tile([128, 4, G4], bf16)
            nc.sync.dma_start(out=wih_sb, in_=wih.rearrange("(k i) c -> i k c", i=128))
            whh_sb = singles.tile([128, 4, G4], fp8)
            nc.gpsimd.dma_start(out=whh_sb, in_=whh8.rearrange("(k i) c -> i k c", i=128))
            bias_sb = singles.tile([128, G4], f32)
            nc.sync.dma_start(out=bias_sb, in_=bias128[:, :])
            eye_sb = singles.tile([16, 16], bf16)
            nc.sync.dma_start(out=eye_sb, in_=eye16[:, :])

            # persistent cell state, one tile per chunk-pair to decouple deps
            c_st = []
            for p in range(2):
                cp = singles.tile([BL, 2, 128], f32, name=f"c_{p}")
                nc.vector.memset(cp, 0.0)
                c_st.append(cp)

            # ---------------- Phase 1: G = x @ W_ihT + bias ----------------
            xTr = xT.rearrange("(k i) c -> i k c", i=128)
            with (
                tc.tile_pool(name="p1psum", bufs=8, space="PSUM") as pp1,
                tc.tile_pool(name="p1x", bufs=3) as xq,
                tc.tile_pool(name="p1g", bufs=6) as gq,
            ):
                for rc in range(T * BL // 128):
                    x_sb = xq.tile([128, 4, 128], bf16, tag="x")
                    nc.sync.dma_start(out=x_sb, in_=xTr[:, :, 128 * rc:128 * (rc + 1)])
                    psums = []
                    for n in range(4):
                        psums.append(pp1.tile([128, 512], f32, tag="pg", name=f"pg_{rc}_{n}"))
                    for k in range(4):
                        for n in range(4):
                            nc.tensor.matmul(
                                psums[n], x_sb[:, k, :],
                                wih_sb[:, k, 512 * n:512 * (n + 1)],
                                start=(k == 0), stop=(k == 3),
                            )
                    for n in range(4):
                        gout = gq.tile([128, 512], bf16, tag="gout")
                        nc.vector.tensor_add(gout, psums[n], bias_sb[:, 512 * n:512 * (n + 1)])
                        nc.sync.dma_start(
                            out=G[8 * rc:8 * (rc + 1), :, 512 * n:512 * (n + 1)],
                            in_=gout,
                        )

            # ---------------- Phase 2: recurrence ----------------
            with (
                tc.tile_pool(name="pgA", bufs=2, space="PSUM") as pgA_pool,
                tc.tile_pool(name="pgB", bufs=1, space="PSUM") as pgB_pool,
                tc.tile_pool(name="ptr", bufs=1, space="PSUM") as pt_pool,
                tc.tile_pool(name="gload", bufs=2) as gl_pool,
                tc.tile_pool(name="obuf", bufs=2) as ob_pool,
                tc.tile_pool(name="work", bufs=3) as wk,
                tc.tile_pool(name="hTp", bufs=2) as hp,
            ):
                hT_prev = None   # [hT01, hT23] of previous step
                gbuf = None
                obuf = None
                for s in range(T):
                    tl = s % GB
                    if tl == 0:
                        gbuf = gl_pool.tile([BL, GB, 4, 512], bf16, tag="gb")
                        nc.gpsimd.dma_start(
                            out=gbuf,
                            in_=G[s:s + GB].rearrange("t b c -> b t c"),
                        )
                        obuf = ob_pool.tile([BL, GB, 4, 128], bf16, tag="ob")

                    # gates PSUM: pair tiles [16, 2, 512] (2 banks each)
                    pg = [
                        pgA_pool.tile([BL, 2, 512], f32, tag="pgA", name=f"pgA_{s}"),
                        pgB_pool.tile([BL, 2, 512], f32, tag="pgB", name=f"pgB_{s}"),
                    ]
                    # G inject (start) then fp8 DoubleRow hh matmuls
                    for p in range(2):
                        for j in range(2):
                            n = 2 * p + j
                            nc.tensor.matmul(
                                pg[p][:, j, :], eye_sb, gbuf[:, tl, n, :],
                                start=True, stop=(s == 0),
                            )
                    if s > 0:
                        for kp in range(2):   # k-pair: chunks (0,1) then (2,3)
                            for p in range(2):
                                for j in range(2):
                                    n = 2 * p + j
                                    nc.tensor.matmul(
                                        pg[p][:, j, :],
                                        hT_prev[kp],
                                        whh_sb[:, 2 * kp:2 * kp + 2, 512 * n:512 * (n + 1)],
                                        start=False, stop=(kp == 1),
                                        perf_mode=DR,
                                    )

                    hT_cur = [
                        hp.tile([128, 2, 16], fp8, tag="hT01", name=f"hT01_{s}"),
                        hp.tile([128, 2, 16], fp8, tag="hT23", name=f"hT23_{s}"),
                    ]
                    for p in range(2):
                        pgp = pg[p]
                        sig = wk.tile([BL, 2, 512], f32, tag=f"sig{p}")
                        nc.scalar.activation(out=sig, in_=pgp, func=SIG)
                        v = wk.tile([BL, 2, 128], f32, tag=f"v{p}")
                        nc.vector.scalar_tensor_tensor(
                            out=v, in0=sig[:, :, 384:512], scalar=0.5,
                            in1=sig[:, :, 0:128], op0=SUB, op1=MUL,
                        )
                        t1 = wk.tile([BL, 2, 128], f32, tag=f"t1{p}")
                        nc.gpsimd.tensor_mul(t1, sig[:, :, 128:256], c_st[p])
                        nc.vector.scalar_tensor_tensor(
                            out=c_st[p], in0=v, scalar=2.0,
                            in1=t1, op0=MUL, op1=ADD,
                        )
                        th = wk.tile([BL, 2, 128], f32, tag=f"th{p}")
                        nc.scalar.activation(out=th, in_=c_st[p], func=TANH)
                        hsl = obuf[:, tl, 2 * p:2 * p + 2, :]
                        nc.gpsimd.tensor_mul(hsl, sig[:, :, 256:384], th)
                        ptr = pt_pool.tile([128, 2, 16], bf16, tag=f"pt{p}", name=f"pt{p}_{s}")
                        for j in range(2):
                            nc.tensor.transpose(ptr[:, j, :], obuf[:, tl, 2 * p + j, :], eye_sb)
                        nc.vector.tensor_copy(out=hT_cur[p], in_=ptr)

                    hT_prev = hT_cur
                    if tl == GB - 1:
                        nc.sync.dma_start(
                            out=out_loc[s - GB + 1:s + 1].rearrange("t b h -> b t h"),
                            in_=obuf,
                        )

    nc.compile()
    return nc


_NC_CACHE = []


def kernel(**inputs):
    from concourse.bass_utils import run_bass_kernel_spmd
    import os

    X = np.asarray(inputs["input"], np.float32)
    perm = _gate_perm()
    gsc = _g_scale()
    eye = np.eye(16, dtype=np.float32)

    in_maps = []
    for c in range(NC):
        d = c // 4
        bs = BL * (c % 4)
        sfx = "f" if d == 0 else "b"
        Wih = np.asarray(inputs[f"W_ih_{sfx}"], np.float32)[perm] * gsc[:, None]
        Whh = np.asarray(inputs[f"W_hh_{sfx}"], np.float32)[perm] * gsc[:, None]
        bias = ((np.asarray(inputs[f"b_ih_{sfx}"], np.float32)
                 + np.asarray(inputs[f"b_hh_{sfx}"], np.float32))[perm] * gsc)
        Xc = X[:, bs:bs + BL, :] if d == 0 else X[::-1, bs:bs + BL, :]
        in_maps.append({
            "xT": np.ascontiguousarray(Xc.transpose(2, 0, 1).reshape(I, T * BL)).astype(ml_dtypes.bfloat16),
            "wih": np.ascontiguousarray(Wih.T).astype(ml_dtypes.bfloat16),
            "whh8": np.ascontiguousarray(Whh.T).astype(ml_dtypes.float8_e4m3fn),
            "bias128": np.ascontiguousarray(np.broadcast_to(bias, (128, G4))).astype(np.float32),
            "eye16": eye.astype(ml_dtypes.bfloat16),
        })

    if not _NC_CACHE:
        _NC_CACHE.append(_build_program())
    nc = _NC_CACHE[0]

    trace = bool(int(os.environ.get("LSTM_TRACE", "0")))
    res = run_bass_kernel_spmd(nc, in_maps, list(range(NC)), trace=trace)
    LAST_RUN_INFO.clear()
    LAST_RUN_INFO["exec_time_ns"] = res.exec_time_ns
    LAST_RUN_INFO["profile_json"] = getattr(res, "profile_json", None)

    out = np.empty((T, B, 2 * H), np.float32)
    for c in range(NC):
        d = c // 4
        bs = BL * (c % 4)
        r = np.asarray(res.results[c]["out_loc"]).astype(np.float32)
        if d == 0:
            out[:, bs:bs + BL, 0:H] = r
        else:
            out[:, bs:bs + BL, H:2 * H] = r[::-1]
    return out


# revision 7
# speedup vs baseline: 1.3461x; 1.1587x over previous
"""Bidirectional LSTM kernel for 8 trn2 NeuronCores.

Problem: T=512, B=64, I=512, H=512, fp32 in/out.
  out[t] = concat(h_f[t], h_b[t]), 2 independent LSTM scans (fwd / bwd).

Sharding: core c in 0..7 -> direction d = c//4 (0=fwd, 1=bwd), batch shard
  bs = 16*(c%4) .. +16.  Backward direction handled purely by host-side data
  prep (time-reversed input; output un-reversed on host).

Per-core plan (SPMD; all cores run the identical program on different data):
  Phase 1: G[t,b,:] = x[t,b,:] @ W_ihT_r + bias   (gate-permuted, bf16 in DRAM)
     stationary = x chunk (128 (t,b) rows), moving = W_ihT (bf16). 64 chunks.
  Phase 2: 512 sequential steps, per step:
     - 4 PSUM banks (chunk-major, layout [i|f|o|g] per 128-hidden chunk),
       ping-pong allocated so next step's matmuls prefetch under this step's
       elementwise.
     - G injected into PSUM via eye-matmul (bf16 moving).
     - recurrent matmuls in fp8e4m3 DoubleRow (2 k-tiles per pass): 8 matmuls
       of [128,2,16] x [128,2,512] per step.
     - g-gate columns pre-scaled x2 on host so tanh(g) = 2*sigmoid(g*2)-1,
       letting one sigmoid cover all 4 gates of a pair.
     - elementwise in 2 chunk-pairs: sigmoid (ACT) -> v=(sg-0.5)*si (DVE STT)
       / t1=sf*c (POOL) -> c=(2v+t1) (DVE STT) -> tanh_c (ACT) -> h=so*th
       (POOL, bf16 out) -> 2 PE transposes -> fp8 cast (DVE) into hT pair.
"""

import numpy as np
import ml_dtypes

T, B, I, H = 512, 64, 512, 512
BL = 16           # batch per core
NC = 8            # cores
GB = 4            # steps per G-load / out-store batch
G4 = 4 * H        # 2048

LAST_RUN_INFO = {}


def _gate_perm():
    # reference gate order after split: i, f, g, o (rows 0:512, 512:1024, 1024:1536, 1536:2048)
    # chunk n layout: [i_n | f_n | o_n | g_n], each 128 wide, hidden slice 128n..128n+128
    idx = []
    for n in range(4):
        s = 128 * n
        idx.extend(range(0 + s, 0 + s + 128))        # i
        idx.extend(range(512 + s, 512 + s + 128))    # f
        idx.extend(range(1536 + s, 1536 + s + 128))  # o
        idx.extend(range(1024 + s, 1024 + s + 128))  # g
    return np.array(idx)


def _g_scale():
    # x2 on the g-gate columns (384:512 of each 512 chunk) for the
    # tanh(g) = 2*sigmoid(2g)-1 fold
    s = np.ones(G4, np.float32)
    for n in range(4):
        s[512 * n + 384:512 * (n + 1)] = 2.0
    return s


def _build_program():
    import concourse.bass as bass
    import concourse.tile as tile
    from concourse import bacc, mybir

    nc = bacc.Bacc()
    f32, bf16 = mybir.dt.float32, mybir.dt.bfloat16
    fp8 = mybir.dt.float8e4
    DR = mybir.MatmulPerfMode.DoubleRow

    xT = nc.declare_dram_parameter("xT", [I, T * BL], bf16, isOutput=False)
    wih = nc.declare_dram_parameter("wih", [I, G4], bf16, isOutput=False)
    whh8 = nc.declare_dram_parameter("whh8", [H, G4], fp8, isOutput=False)
    bias128 = nc.declare_dram_parameter("bias128", [128, G4], f32, isOutput=False)
    eye16 = nc.declare_dram_parameter("eye16", [16, 16], bf16, isOutput=False)
    out_loc = nc.declare_dram_parameter("out_loc", [T, BL, H], bf16, isOutput=True)

    G = nc.dram_tensor("G_scratch", [T, BL, G4], bf16)

    SIG = mybir.ActivationFunctionType.Sigmoid
    TANH = mybir.ActivationFunctionType.Tanh
    MUL = mybir.AluOpType.mult
    ADD = mybir.AluOpType.add
    SUB = mybir.AluOpType.subtract

    with tile.TileContext(nc) as tc:
        from contextlib import ExitStack
        with ExitStack() as ctx:
            singles = ctx.enter_context(tc.tile_pool(name="singles", bufs=1))

            wih_sb = singles.tile([128, 4, G4], bf16)
            nc.sync.dma_start(out=wih_sb, in_=wih.rearrange("(k i) c -> i k c", i=128))
            whh_sb = singles.tile([128, 4, G4], fp8)
            nc.gpsimd.dma_start(out=whh_sb, in_=whh8.rearrange("(k i) c -> i k c", i=128))
            bias_sb = singles.tile([128, G4], f32)
            nc.sync.dma_start(out=bias_sb, in_=bias128[:, :])
            eye_sb = singles.tile([16, 16], bf16)
            nc.sync.dma_start(out=eye_sb, in_=eye16[:, :])

            # persistent cell state, one tile per chunk-pair to decouple deps
            c_st = []
            for p in range(2):
                cp = singles.tile([BL, 2, 128], bf16, name=f"c_{p}")
                nc.vector.memset(cp, 0.0)
                c_st.append(cp)

            # ---------------- Phase 1: G = x @ W_ihT + bias ----------------
            xTr = xT.rearrange("(k i) c -> i k c", i=128)
            with (
                tc.tile_pool(name="p1psum", bufs=8, space="PSUM") as pp1,
                tc.tile_pool(name="p1x", bufs=3) as xq,
                tc.tile_pool(name="p1g", bufs=6) as gq,
            ):
                for rc in range(T * BL // 128):
                    x_sb = xq.tile([128, 4, 128], bf16, tag="x")
                    nc.sync.dma_start(out=x_sb, in_=xTr[:, :, 128 * rc:128 * (rc + 1)])
                    psums = []
                    for n in range(4):
                        psums.append(pp1.tile([128, 512], f32, tag="pg", name=f"pg_{rc}_{n}"))
                    for k in range(4):
                        for n in range(4):
                            nc.tensor.matmul(
                                psums[n], x_sb[:, k, :],
                                wih_sb[:, k, 512 * n:512 * (n + 1)],
                                start=(k == 0), stop=(k == 3),
                            )
                    for n in range(4):
                        gout = gq.tile([128, 512], bf16, tag="gout")
                        nc.vector.tensor_add(gout, psums[n], bias_sb[:, 512 * n:512 * (n + 1)])
                        nc.sync.dma_start(
                            out=G[8 * rc:8 * (rc + 1), :, 512 * n:512 * (n + 1)],
                            in_=gout,
                        )

            # ---------------- Phase 2: recurrence ----------------
            with (
                tc.tile_pool(name="pgA", bufs=2, space="PSUM") as pgA_pool,
                tc.tile_pool(name="pgB", bufs=1, space="PSUM") as pgB_pool,
                tc.tile_pool(name="ptr", bufs=1, space="PSUM") as pt_pool,
                tc.tile_pool(name="gload", bufs=2) as gl_pool,
                tc.tile_pool(name="obuf", bufs=2) as ob_pool,
                tc.tile_pool(name="work", bufs=3) as wk,
                tc.tile_pool(name="hTp", bufs=2) as hp,
            ):
                hT_prev = None   # [hT01, hT23] of previous step
                gbuf = None
                obuf = None
                for s in range(T):
                    tl = s % GB
                    if tl == 0:
                        gbuf = gl_pool.tile([BL, GB, 4, 512], bf16, tag="gb")
                        nc.sync.dma_start(
                            out=gbuf,
                            in_=G[s:s + GB].rearrange("t b c -> b t c"),
                        )
                        obuf = ob_pool.tile([BL, GB, 4, 128], bf16, tag="ob")

                    # gates PSUM: pair tiles [16, 2, 512] (2 banks each)
                    pg = [
                        pgA_pool.tile([BL, 2, 512], f32, tag="pgA", name=f"pgA_{s}"),
                        pgB_pool.tile([BL, 2, 512], f32, tag="pgB", name=f"pgB_{s}"),
                    ]
                    # G inject (start) then fp8 DoubleRow hh matmuls
                    for p in range(2):
                        for j in range(2):
                            n = 2 * p + j
                            nc.tensor.matmul(
                                pg[p][:, j, :], eye_sb, gbuf[:, tl, n, :],
                                start=True, stop=(s == 0),
                            )
                    if s > 0:
                        for kp in range(2):   # k-pair: chunks (0,1) then (2,3)
                            for p in range(2):
                                for j in range(2):
                                    n = 2 * p + j
                                    nc.tensor.matmul(
                                        pg[p][:, j, :],
                                        hT_prev[kp],
                                        whh_sb[:, 2 * kp:2 * kp + 2, 512 * n:512 * (n + 1)],
                                        start=False, stop=(kp == 1),
                                        perf_mode=DR,
                                    )

                    hT_cur = [
                        hp.tile([128, 2, 16], fp8, tag="hT01", name=f"hT01_{s}"),
                        hp.tile([128, 2, 16], fp8, tag="hT23", name=f"hT23_{s}"),
                    ]
                    for p in range(2):
                        pgp = pg[p]
                        sig = wk.tile([BL, 2, 512], bf16, tag=f"sig{p}")
                        nc.scalar.activation(out=sig, in_=pgp, func=SIG)
                        v = wk.tile([BL, 2, 128], bf16, tag=f"v{p}")
                        nc.vector.scalar_tensor_tensor(
                            out=v, in0=sig[:, :, 384:512], scalar=0.5,
                            in1=sig[:, :, 0:128], op0=SUB, op1=MUL,
                        )
                        t1 = wk.tile([BL, 2, 128], bf16, tag=f"t1{p}")
                        nc.vector.tensor_mul(t1, sig[:, :, 128:256], c_st[p])
                        nc.vector.scalar_tensor_tensor(
                            out=c_st[p], in0=v, scalar=2.0,
                            in1=t1, op0=MUL, op1=ADD,
                        )
                        th = wk.tile([BL, 2, 128], bf16, tag=f"th{p}")
                        nc.scalar.activation(out=th, in_=c_st[p], func=TANH)
                        hsl = obuf[:, tl, 2 * p:2 * p + 2, :]
                        nc.vector.tensor_mul(hsl, sig[:, :, 256:384], th)
                        ptr = pt_pool.tile([128, 2, 16], bf16, tag=f"pt{p}", name=f"pt{p}_{s}")
                        for j in range(2):
                            nc.tensor.transpose(ptr[:, j, :], obuf[:, tl, 2 * p + j, :], eye_sb)
                        nc.vector.tensor_copy(out=hT_cur[p], in_=ptr)

                    hT_prev = hT_cur
                    if tl == GB - 1:
                        nc.sync.dma_start(
                            out=out_loc[s - GB + 1:s + 1].rearrange("t b h -> b t h"),
                            in_=obuf,
                        )

    nc.compile()
    return nc


_NC_CACHE = []


def kernel(**inputs):
    from concourse.bass_utils import run_bass_kernel_spmd
    import os

    X = np.asarray(inputs["input"], np.float32)
    perm = _gate_perm()
    gsc = _g_scale()
    eye = np.eye(16, dtype=np.float32)

    in_maps = []
    for c in range(NC):
        d = c // 4
        bs = BL * (c % 4)
        sfx = "f" if d == 0 else "b"
        Wih = np.asarray(inputs[f"W_ih_{sfx}"], np.float32)[perm] * gsc[:, None]
        Whh = np.asarray(inputs[f"W_hh_{sfx}"], np.float32)[perm] * gsc[:, None]
        bias = ((np.asarray(inputs[f"b_ih_{sfx}"], np.float32)
                 + np.asarray(inputs[f"b_hh_{sfx}"], np.float32))[perm] * gsc)
        Xc = X[:, bs:bs + BL, :] if d == 0 else X[::-1, bs:bs + BL, :]
        in_maps.append({
            "xT": np.ascontiguousarray(Xc.transpose(2, 0, 1).reshape(I, T * BL)).astype(ml_dtypes.bfloat16),
            "wih": np.ascontiguousarray(Wih.T).astype(ml_dtypes.bfloat16),
            "whh8": np.ascontiguousarray(Whh.T).astype(ml_dtypes.float8_e4m3fn),
            "bias128": np.ascontiguousarray(np.broadcast_to(bias, (128, G4))).astype(np.float32),
            "eye16": eye.astype(ml_dtypes.bfloat16),
        })

    if not _NC_CACHE:
        _NC_CACHE.append(_build_program())
    nc = _NC_CACHE[0]

    trace = bool(int(os.environ.get("LSTM_TRACE", "0")))
    res = run_bass_kernel_spmd(nc, in_maps, list(range(NC)), trace=trace)
    LAST_RUN_INFO.clear()
    LAST_RUN_INFO["exec_time_ns"] = res.exec_time_ns
    LAST_RUN_INFO["profile_json"] = getattr(res, "profile_json", None)

    out = np.empty((T, B, 2 * H), np.float32)
    for c in range(NC):
        d = c // 4
        bs = BL * (c % 4)
        r = np.asarray(res.results[c]["out_loc"]).astype(np.float32)
        if d == 0:
            out[:, bs:bs + BL, 0:H] = r
        else:
            out[:, bs:bs + BL, H:2 * H] = r[::-1]
    return out
